# revision 84
# baseline (speedup 1.0000x reference)
"""
Distributed GQA attention block for Trainium2 (8 NeuronCores).

Problem: out = AttentionBlock(x; wq, wk, wv, wo)
  B=2, S=2048, DIM=4096, n_heads=32, n_kv_heads=8, head_dim=128,
  rope theta=5e5, causal, softmax, f32 I/O.

Sharding strategy (tensor-parallel over heads, pipelined ReduceScatter
after a ROW-parallel output projection):
  - Each core c owns 4 query heads (4c..4c+3) and 1 kv head (c).
  - Per core: q/k/v projections for its heads (column shards of wq/wk/wv),
    RoPE, causal attention for its 4 heads over the full sequence. The
    per-core attention output stays in SBUF, stored transposed
    [512 feat, tokens] in bf16.
  - The output projection is ROW-sharded: core c computes
    partial = attnT_c^T @ wo[512c:512c+512, :]  -> [tokens, 4096] bf16,
    streamed to DRAM in 512-token groups. Each group is ReduceScattered
    (sum over the 8 cores, scatter over tokens) into the kernel output:
    core c receives tokens [g*512 + 64c : g*512 + 64c + 64) of group g.
    Host-side unshard is pure index assembly along tokens (no compute).
  - The 8 small ReduceScatters pipeline behind the projection compute, so
    only the last (~28us) is exposed, vs ~330us of exposed AllGather time
    in the gather + column-parallel-wo formulation.

Compute dtype: bf16 operands with f32 PSUM accumulation. Softmax skips
the max-subtraction (scores are < ~15 at this problem's scale), the
denominator comes free from an appended ones-column in the PV matmul,
and normalization is applied to the [tok, 128] attention output instead
of the [tok, 2048] probabilities.

RoPE layout trick: wq/wk columns are host-permuted so each head's even
dims come first and odd dims second. The rotation's pair swap then
becomes two 64-partition block copies (SBUF->SBUF DMA) instead of a
cross-partition interleave.
"""

import math
from types import SimpleNamespace

import numpy as np
import ml_dtypes

P = 128
BF16 = ml_dtypes.bfloat16


_CACHE = {}
_TRACE = False


def make_cfg(B=2, S=2048, DIM=4096, H=32, KVH=8, HD=128, THETA=500000.0,
             NCORES=8):
    c = SimpleNamespace(B=B, S=S, DIM=DIM, H=H, KVH=KVH, HD=HD, THETA=THETA,
                        NCORES=NCORES)
    c.T = B * S
    c.HPC = H // NCORES          # query heads per core
    c.QF = c.HPC * HD            # query features per core
    c.SCALE = 1.0 / math.sqrt(HD)
    c.TCH = 512                  # token chunk
    c.NKT = DIM // P             # contraction tiles
    c.NTT = c.T // P             # token tiles
    c.NCH = c.T // c.TCH         # token chunks (= ReduceScatter groups)
    c.SQT = S // P               # q/k tiles per sequence
    c.VW = HD + 1                # v + ones column
    c.OSH = c.TCH // NCORES      # output token rows per core per group
    assert S % c.TCH == 0 and c.T % c.TCH == 0 and DIM % P == 0
    # each core's HPC query heads share the core's single kv head
    assert KVH == NCORES and c.HPC == H // KVH
    return c


def _build_graph(c, phases=4):
    """Build + compile the SPMD Bass graph (same program on every core)."""
    import concourse.mybir as mybir
    import concourse.tile as tile
    from concourse import bacc

    fp32 = mybir.dt.float32
    bf16 = mybir.dt.bfloat16

    nc = bacc.Bacc(
        "TRN2",
        target_bir_lowering=False,
        debug=False,
        enable_asserts=True,
        num_devices=c.NCORES,
    )

    # ---- kernel I/O ----
    xT = nc.dram_tensor("xT", [c.DIM, c.T], bf16, kind="ExternalInput").ap()
    wq = nc.dram_tensor("wq", [c.DIM, c.QF], bf16, kind="ExternalInput").ap()
    wk = nc.dram_tensor("wk", [c.DIM, c.HD], bf16, kind="ExternalInput").ap()
    wv = nc.dram_tensor("wv", [c.DIM, c.HD], bf16, kind="ExternalInput").ap()
    wo = nc.dram_tensor("wo", [c.QF, c.DIM], bf16, kind="ExternalInput").ap()
    cosi = nc.dram_tensor("cosi", [P, c.T], fp32, kind="ExternalInput").ap()
    sini = nc.dram_tensor("sini", [P, c.T], fp32, kind="ExternalInput").ap()
    tril = nc.dram_tensor("tril", [P, P], bf16, kind="ExternalInput").ap()
    ident = nc.dram_tensor("ident", [P, P], fp32, kind="ExternalInput").ap()
    # token-sharded output: group g of 512 tokens contributes rows
    # [g*64, g*64+64) = global tokens [g*512 + 64*rank, +64)
    out = nc.dram_tensor("out", [c.NCH * c.OSH, c.DIM], bf16,
                         kind="ExternalOutput").ap()

    Exp = mybir.ActivationFunctionType.Exp
    Copy = mybir.ActivationFunctionType.Copy
    TPP = c.TCH // P          # token sub-tiles per chunk
    NQT = c.HPC + 1           # rope targets per chunk: HPC q tiles + 1 k
    SPB = c.S // P            # 128-token tiles per batch
    CPB = c.NCH // c.B        # token chunks per batch
    KG = 4                    # contraction tiles fetched per DMA
    NOC = c.DIM // c.TCH      # output column chunks (phase 4)

    with tile.TileContext(nc) as tc:
        # ------- static SBUF tensors (split per batch so attention of
        # batch 0 can start while batch 1 is still projecting) -------
        qT_b, kT_b, v_b, free_stat = [], [], [], []
        for b in range(c.B):
            t_, f_ = tc.tile([P, c.HPC, c.S], bf16, name=f"qT_sb{b}")
            qT_b.append(t_); free_stat.append(f_)
            t_, f_ = tc.tile([P, c.S], bf16, name=f"kT_sb{b}")
            kT_b.append(t_); free_stat.append(f_)
            t_, f_ = tc.tile([P, SPB, c.VW], bf16, name=f"v_sb{b}")
            v_b.append(t_); free_stat.append(f_)
        tril_sb, free_tril = tc.tile([P, P], bf16, name="tril_sb")
        id_sb, free_id = tc.tile([P, P], fp32, name="id_sb")
        idb_sb, free_idb = tc.tile([P, P], bf16, name="idb_sb")
        free_stat += [free_tril, free_id, free_idb]

        for b in range(c.B):
            nc.vector.memset(v_b[b][:, :, c.HD:c.VW], 1.0)  # denominator ones

        # dummy exp at t=0: pulls the ~2.7us exp_and_others ACT-table load
        # off the attention critical path (Copy is filler in every set, so
        # no switch-back occurs later). Sourced from the memset region so it
        # does not wait on any DMA.
        warm_sb, free_warm = tc.tile([1, 1], fp32, name="warm_sb")
        nc.scalar.activation(warm_sb[:], v_b[0][0:1, 0, c.HD:c.HD + 1], Exp)
        free_stat.append(free_warm)

        with tc.tile_pool(name="dram", bufs=1, space="DRAM") as dramp:
            # per-group partial output-projection products (RS inputs)
            part_g = [
                dramp.tile([c.TCH, c.DIM], bf16, name=f"part{g}")
                for g in range(c.NCH)
            ]
            # RS outputs (collectives may not write IO tensors directly)
            rs_g = [
                dramp.tile([c.OSH, c.DIM], bf16, name=f"rs{g}")
                for g in range(c.NCH)
            ]

            # ============ Phase 1: projections + RoPE ============
            WG = 8                    # wk/wv contraction tiles per DMA
            with tc.tile_pool(name="wpool", bufs=1) as wpool, \
                 tc.tile_pool(name="xpool", bufs=3) as xpool, \
                 tc.tile_pool(name="tabs", bufs=2) as tabs, \
                 tc.tile_pool(name="rope", bufs=2) as ropep, \
                 tc.tile_pool(name="pj_ps", bufs=1, space="PSUM") as pjps:

                # per-kt weight tiles: wq rides the fast sync/HWDGE queue
                # interleaved just-in-time with the x stream (4-tile
                # mega-DMAs); wk/wv go on the gpsimd queue in 8-tile
                # mega-DMAs (64 separate small DMAs can't keep up with the
                # matmul stream's 1.28us/kt)
                wqb = [None] * (c.NKT // KG)
                wkb, wvb = [], []
                for gi in range(c.NKT // WG):
                    r0, r1 = gi * WG * P, (gi + 1) * WG * P
                    wkt = wpool.tile([P, WG, c.HD], bf16, tag="wk",
                                     bufs=c.NKT // WG, name=f"wk_g{gi}")
                    wvt = wpool.tile([P, WG, c.HD], bf16, tag="wv",
                                     bufs=c.NKT // WG, name=f"wv_g{gi}")
                    if gi == 0:
                        # split head: the first 2 k/v tiles land fast so
                        # the t=0 matmuls are not gated on an 8-tile DMA
                        rm = r0 + 2 * P
                        nc.gpsimd.dma_start(
                            wkt[:, 0:2, :],
                            wk[r0:rm, :].rearrange("(o p) h -> p o h", p=P))
                        nc.gpsimd.dma_start(
                            wvt[:, 0:2, :],
                            wv[r0:rm, :].rearrange("(o p) h -> p o h", p=P))
                        nc.gpsimd.dma_start(
                            wkt[:, 2:WG, :],
                            wk[rm:r1, :].rearrange("(o p) h -> p o h", p=P))
                        nc.gpsimd.dma_start(
                            wvt[:, 2:WG, :],
                            wv[rm:r1, :].rearrange("(o p) h -> p o h", p=P))
                    else:
                        nc.gpsimd.dma_start(
                            wkt[:],
                            wk[r0:r1, :].rearrange("(o p) h -> p o h", p=P))
                        nc.gpsimd.dma_start(
                            wvt[:],
                            wv[r0:r1, :].rearrange("(o p) h -> p o h", p=P))
                    wkb.append(wkt)
                    wvb.append(wvt)

                def wk_at(kt):
                    return wkb[kt // WG][:, kt % WG, :]

                def wv_at(kt):
                    return wvb[kt // WG][:, kt % WG, :]

                def load_wq(kg):
                    wqt = wpool.tile([P, KG, c.QF], bf16, tag="wq",
                                     bufs=c.NKT // KG, name=f"wq_g{kg}")
                    r0 = kg * KG * P
                    if kg == 0:
                        nc.sync.dma_start(
                            wqt[:, 0:1, :],
                            wq[r0:r0 + P, :].rearrange(
                                "(o p) f -> p o f", p=P))
                        nc.sync.dma_start(
                            wqt[:, 1:KG, :],
                            wq[r0 + P:r0 + KG * P, :].rearrange(
                                "(o p) f -> p o f", p=P))
                    else:
                        nc.sync.dma_start(
                            wqt[:],
                            wq[r0:r0 + KG * P, :].rearrange(
                                "(o p) f -> p o f", p=P))
                    wqb[kg] = wqt

                def wq_at(kt, ft):
                    return wqb[kt // KG][:, kt % KG, ft * P:(ft + 1) * P]

                for ch in range(c.NCH):
                    t0 = ch * c.TCH
                    bch = ch // CPB           # batch of this chunk
                    lt0 = t0 - bch * c.S      # batch-local token offset
                    q_ps = [
                        pjps.tile([P, c.TCH], fp32, tag=f"q{ft}", bufs=1,
                                  name=f"q_ps{ft}")
                        for ft in range(c.HPC)
                    ]
                    k_ps = pjps.tile([P, c.TCH], fp32, tag="k", bufs=1)
                    v_ps = pjps.tile([P, TPP, P], fp32, tag="v", bufs=1)

                    for kg in range(c.NKT // KG):
                        # one DMA brings KG=4 contraction tiles (512 KB)
                        xt4 = xpool.tile([P, KG, c.TCH], bf16, tag="xt")
                        r0 = kg * KG * P
                        if ch == 0 and kg == 0:
                            # split head for a fast t=0 start
                            nc.sync.dma_start(
                                xt4[:, 0:1, :],
                                xT[r0:r0 + P, t0:t0 + c.TCH].rearrange(
                                    "(o p) t -> p o t", p=P))
                            nc.sync.dma_start(
                                xt4[:, 1:KG, :],
                                xT[r0 + P:r0 + KG * P,
                                   t0:t0 + c.TCH].rearrange(
                                    "(o p) t -> p o t", p=P))
                        else:
                            nc.sync.dma_start(
                                xt4[:],
                                xT[r0:r0 + KG * P, t0:t0 + c.TCH].rearrange(
                                    "(o p) t -> p o t", p=P))
                        if ch == 0:
                            # wq arrives just behind the x tile it multiplies
                            load_wq(kg)
                        if ch == 0 and kg == 0:
                            # small static loads ride behind the first
                            # wq/x tiles so the first matmul starts sooner
                            nc.sync.dma_start(tril_sb[:], tril[:])
                            nc.sync.dma_start(id_sb[:], ident[:])
                            nc.vector.tensor_copy(idb_sb[:], id_sb[:])
                        for ki in range(KG):
                            kt = kg * KG + ki
                            xt = xt4[:, ki, :]
                            st = kt == 0
                            sp = kt == c.NKT - 1

                            def mm_kv():
                                nc.tensor.matmul(
                                    k_ps[:], lhsT=wk_at(kt), rhs=xt,
                                    start=st, stop=sp,
                                )
                                # v TOKEN-major: the x tile is lhsT, so no
                                # PE transposes are needed afterwards (same
                                # row count: 4x128 free vs 1x512)
                                for sub in range(TPP):
                                    # start zeroes the whole PSUM bank, so
                                    # only the first of the four region
                                    # chains may issue it
                                    nc.tensor.matmul(
                                        v_ps[:, sub, :],
                                        lhsT=xt4[:, ki,
                                                 sub * P:(sub + 1) * P],
                                        rhs=wv_at(kt),
                                        start=(st and sub == 0), stop=sp,
                                    )

                            if ch == 0:
                                # k/v weights land first (gpsimd queue), so
                                # at t=0 their matmuls lead the q ones
                                mm_kv()
                            for ft in range(c.HPC):
                                nc.tensor.matmul(
                                    q_ps[ft][:],
                                    lhsT=wq_at(kt, ft),
                                    rhs=xt,
                                    start=st, stop=sp,
                                )
                            if ch != 0:
                                mm_kv()

                    # ---- RoPE on all q tiles + k at once (mega-tile) ----
                    ct = tabs.tile([P, c.TCH], fp32, tag="cos")
                    st_t = tabs.tile([P, c.TCH], fp32, tag="sin")
                    nc.sync.dma_start(ct[:], cosi[:, t0:t0 + c.TCH])
                    nc.sync.dma_start(st_t[:], sini[:, t0:t0 + c.TCH])

                    qbig = ropep.tile([P, NQT, c.TCH], fp32, tag="qbig",
                                      name="qbig")
                    # psum -> sbuf copies split across ACT and DVE, ordered
                    # by the next chunk's accumulator-reuse order (q0 first,
                    # v last) so the next chunk's matmuls never wait:
                    #   ACT: q0, q2, k, v0, v2     DVE: q1, q3, v1, v3
                    gt0 = lt0 // P
                    nc.scalar.activation(qbig[:, 0, :], q_ps[0][:], Copy)
                    nc.vector.tensor_copy(qbig[:, 1, :], q_ps[1][:])
                    nc.scalar.activation(qbig[:, 2, :], q_ps[2][:], Copy)
                    nc.vector.tensor_copy(qbig[:, 3, :], q_ps[3][:])
                    nc.scalar.activation(qbig[:, c.HPC, :], k_ps[:], Copy)
                    nc.scalar.activation(v_b[bch][:, gt0, 0:c.HD],
                                         v_ps[:, 0, :], Copy)
                    nc.vector.tensor_copy(v_b[bch][:, gt0 + 1, 0:c.HD],
                                          v_ps[:, 1, :])
                    nc.scalar.activation(v_b[bch][:, gt0 + 2, 0:c.HD],
                                         v_ps[:, 2, :], Copy)
                    nc.vector.tensor_copy(v_b[bch][:, gt0 + 3, 0:c.HD],
                                          v_ps[:, 3, :])

                    qsw = ropep.tile([P, NQT, c.TCH], fp32, tag="qsw",
                                     name="qsw")
                    # pair swap == half-partition block swap (even|odd split)
                    nc.sync.dma_start(qsw[0:64, :, :], qbig[64:128, :, :])
                    nc.sync.dma_start(qsw[64:128, :, :], qbig[0:64, :, :])

                    ctb = ct[:, None, :].to_broadcast((P, NQT, c.TCH))
                    stb = st_t[:, None, :].to_broadcast((P, NQT, c.TCH))
                    eng = nc.vector
                    eng.tensor_mul(qbig[:], qbig[:], ctb)
                    eng.tensor_mul(qsw[:], qsw[:], stb)
                    rr = ropep.tile([P, NQT, c.TCH], bf16, tag="rr", name="rr")
                    eng.tensor_add(rr[:], qbig[:], qsw[:])
                    for ft in range(c.HPC):
                        eng.tensor_copy(
                            qT_b[bch][:, ft, lt0:lt0 + c.TCH], rr[:, ft, :])
                    eng.tensor_copy(
                        kT_b[bch][:, lt0:lt0 + c.TCH], rr[:, c.HPC, :])


            # ====== Phases 2+4 per batch: attention -> wo -> RS ======
            if phases >= 2:
                with tc.tile_pool(name="spool", bufs=3, space="PSUM") as spool, \
                     tc.tile_pool(name="opool", bufs=4, space="PSUM") as opool, \
                     tc.tile_pool(name="tpool", bufs=1, space="PSUM") as tpool, \
                     tc.tile_pool(name="ppool", bufs=32) as ppool, \
                     tc.tile_pool(name="apool", bufs=4) as apool, \
                     tc.tile_pool(name="wop", bufs=1) as wop, \
                     tc.tile_pool(name="atp", bufs=2) as atp, \
                     tc.tile_pool(name="obp", bufs=2) as obp:

                    # wo row-shard [QF, DIM] as 4 feature-partition tiles;
                    # loaded on the gpsimd queue during batch-0 attention
                    wo_t = []
                    for f in range(c.HPC):
                        wot = wop.tile([P, c.DIM], bf16, tag="wo",
                                       bufs=c.HPC, name=f"wo_t{f}")
                        nc.gpsimd.dma_start(wot[:], wo[f * P:(f + 1) * P, :])
                        wo_t.append(wot)

                    def finalize(pend):
                        # PE transpose of the normalized attention tile +
                        # copy into the SBUF attnT store. Deferred one step
                        # behind the main loop so the transpose never waits
                        # on the DVE normalize chain. Copies go to DVE: the
                        # ACT engine is saturated by the exp stream.
                        p_attnT, p_h, p_qi, p_ao = pend
                        tp = tpool.tile([P, P], bf16, tag="tp", name="tp")
                        nc.tensor.transpose(tp[:], p_ao[:], idb_sb[:])
                        nc.vector.tensor_copy(
                            p_attnT[:, p_h, p_qi * P:(p_qi + 1) * P], tp[:])

                    def emit_last_pv(pend):
                        # deferred final PV accumulation steps (the ones
                        # consuming the freshest exp outputs) + normalize.
                        # Emitted after the NEXT unit's scores so PE work
                        # covers the ACT exp latency.
                        p_ops, p_steps, p_start, p_qi, p_attnT, p_h = pend
                        for i, (p_pt, p_off, p_vt) in enumerate(p_steps):
                            nc.tensor.matmul(
                                p_ops, lhsT=p_pt[:, p_off:p_off + P],
                                rhs=p_vt,
                                start=False,
                                stop=(i == len(p_steps) - 1),
                            )
                        rec = apool.tile([P, 1], fp32, tag="rec", name="rec")
                        nc.vector.reciprocal(rec[:], p_ops[:, c.HD:c.VW])
                        # normalize + cast bf16 in one DVE op; the bf16
                        # operand makes the PE transpose 1 cyc/row
                        ao = apool.tile([P, P], bf16, tag="ao", name="ao")
                        nc.vector.tensor_scalar_mul(
                            ao[:], p_ops[:, 0:c.HD], rec[:]
                        )
                        # re-zero for the chain that reuses this region
                        # three units from now (start=False accumulation)
                        nc.vector.memset(p_ops, 0.0)
                        return (p_attnT, p_h, p_qi, ao)

                    pending_o = None
                    pq = []    # deferred last-PV queue (depth 2)
                    # single-bank 3-region PV accumulator. All chains use
                    # start=False onto explicitly zeroed regions: start=True
                    # zeroes the WHOLE bank, clobbering the neighbor chain
                    # that is still open under the 2-deep deferral.
                    o_pp = opool.tile([P, 3, c.VW], fp32, tag="o", bufs=1,
                                      name="o_pp")
                    nc.vector.memset(o_pp[:], 0.0)
                    n_ki = 0
                    SC = 1024             # score tile width (one exp per tile)
                    def score_tile(b, qh, kj, c0, w, tril):
                        s_ps = spool.tile([P, SC], fp32, tag="s",
                                          name="s_ps")
                        for sub in range(0, w, c.TCH):
                            ws = min(c.TCH, w - sub)
                            nc.tensor.matmul(
                                s_ps[:, sub:sub + ws],
                                lhsT=kT_b[b][:, kj * P:(kj + 1) * P],
                                rhs=qh[:, c0 + sub:c0 + sub + ws],
                                start=True, stop=True,
                            )
                        pt = ppool.tile([P, SC], bf16, tag="pt", bufs=28,
                                        name="pt")
                        nc.scalar.activation(pt[:, :w], s_ps[:, :w], Exp,
                                             scale=c.SCALE)
                        if tril:
                            nc.vector.tensor_mul(pt[:, 0:P], pt[:, 0:P],
                                                 tril_sb[:])
                        return pt

                    def emit_unit(b, h, ki, attnT, pts):
                        # one (head, k-tile) attention step: fresh scores,
                        # previous deferred work, this qi's PV chain
                        nonlocal pending_o, n_ki
                        qh = qT_b[b][:, h, :]
                        q0 = ki * P
                        nmc = (c.S - q0 + SC - 1) // SC
                        for mc in range(nmc):
                            c0 = q0 + mc * SC
                            pts[(ki, mc)] = score_tile(
                                b, qh, ki, c0, min(SC, c.S - c0), mc == 0)
                        if len(pq) >= 2:
                            old_fin = pending_o
                            pending_o = emit_last_pv(pq.pop(0))
                            if old_fin is not None:
                                finalize(old_fin)
                        # ---- PV for qi == ki (all but the last step;
                        # that one is deferred TWO units back) ----
                        qi = ki
                        o_ps = o_pp[:, n_ki % 3, :]
                        n_ki += 1
                        for kj in range(qi):
                            qoff = (qi - kj) * P
                            mc = qoff // SC
                            off = qoff % SC
                            nc.tensor.matmul(
                                o_ps,
                                lhsT=pts[(kj, mc)][:, off:off + P],
                                rhs=v_b[b][:, kj, :],
                                start=False, stop=False,
                            )
                        pq.append((o_ps,
                                   [(pts[(qi, 0)], 0, v_b[b][:, qi, :])],
                                   qi == 0, qi, attnT, h))

                    attnTs = [
                        atp.tile([P, c.HPC, c.S], bf16, tag="attnT",
                                 name=f"attnT{bb}")
                        for bb in range(c.B)
                    ]
                    pts_next = {}  # batch-1 head-0 tiles (built inside P4 b0)
                    pts_next2 = {}  # batch-1 head-1 tiles (ditto)
                    pts_next3 = {}  # batch-1 head-2 tiles (ditto)

                    def p4_batch(b, attnT, extra):
                        # row-parallel wo + pipelined RS; `extra` holds one
                        # attention-unit thunk per token tile (the next
                        # batch's first head rides the idle ACT engine here)
                        nonlocal pending_o
                        ei = 0
                        for gl in range(CPB):         # 512-token groups
                            g = b * CPB + gl
                            for tt in range(TPP):     # 128-token tiles
                                lt = gl * c.TCH + tt * P
                                ob = obp.tile([P, c.DIM], bf16, tag="ob",
                                              bufs=3, name="ob")
                                for cc in range(c.DIM // SC):
                                    o4 = spool.tile([P, SC], fp32,
                                                    tag="s", name="o4_ps")
                                    for half in range(SC // c.TCH):
                                        hb = half * c.TCH
                                        for f in range(c.HPC):
                                            nc.tensor.matmul(
                                                o4[:, hb:hb + c.TCH],
                                                lhsT=attnT[:, f, lt:lt + P],
                                                rhs=wo_t[f][:,
                                                            cc * SC + hb:
                                                            cc * SC + hb
                                                            + c.TCH],
                                                start=(f == 0),
                                                stop=(f == c.HPC - 1),
                                            )
                                    if pq:
                                        old_fin = pending_o
                                        pending_o = emit_last_pv(pq.pop(0))
                                        if old_fin is not None:
                                            finalize(old_fin)
                                    elif pending_o is not None:
                                        finalize(pending_o)
                                        pending_o = None
                                    dst = ob[:, cc * SC:(cc + 1) * SC]
                                    if cc % 2 == 0:
                                        nc.scalar.activation(dst, o4[:], Copy)
                                    else:
                                        nc.vector.tensor_copy(dst, o4[:])
                                # on the gpsimd dma queue: the sync queue is
                                # reserved for rs->out copies (which wait on
                                # a RS) and the ACT/DVE queues must keep
                                # draining PSUM copies, so partial writes
                                # never block anything
                                nc.gpsimd.dma_start(
                                    part_g[g][tt * P:(tt + 1) * P, :], ob[:]
                                )
                                for _ in range(3):
                                    if ei < len(extra):
                                        extra[ei]()
                                        ei += 1
                            if phases >= 3:
                                # sum over cores, scatter over this group's
                                # tokens: rank r receives rows [64r, 64r+64)
                                nc.gpsimd.collective_compute(
                                    "ReduceScatter",
                                    mybir.AluOpType.add,
                                    replica_groups=[list(range(c.NCORES))],
                                    ins=[part_g[g][:].opt()],
                                    outs=[rs_g[g][:].opt()],
                                )
                                nc.sync.dma_start(
                                    out[g * c.OSH:(g + 1) * c.OSH, :],
                                    rs_g[g][:],
                                )
                        assert ei == len(extra)

                    for b in range(c.B):
                        # -------- attention for this batch's heads -------
                        # (batch 1's head 0 already ran inside P4 of b0)
                        attnT = attnTs[b]
                        for h in range(3 if b > 0 else 0, c.HPC):
                            pts = {}
                            for ki in range(c.SQT):
                                emit_unit(b, h, ki, attnT, pts)
                        if b == 0:
                            extra = [
                                (lambda kk=kk, hh=hh, pp=pp: emit_unit(
                                    1, hh, kk, attnTs[1], pp))
                                for hh, pp in ((0, pts_next), (1, pts_next2),
                                               (2, pts_next3))
                                for kk in range(c.SQT)
                            ]
                        else:
                            extra = []
                        p4_batch(b, attnT, extra)

        # release static single-tile pools in LIFO order
        for f_ in reversed(free_stat):
            f_()

    nc.compile()
    return nc


def _host_inputs(c, x, wq, wk, wv, wo):
    """Shard + lay out the inputs for the cores."""
    xT = np.ascontiguousarray(x.reshape(c.T, c.DIM).T).astype(BF16)

    # even/odd split permutation within each head (q and k only)
    perm_head = np.concatenate([np.arange(0, c.HD, 2), np.arange(1, c.HD, 2)])

    def permute_heads(w):  # w: [DIM, n*HD]
        nh = w.shape[1] // c.HD
        w = w.reshape(c.DIM, nh, c.HD)[:, :, perm_head]
        return np.ascontiguousarray(w.reshape(c.DIM, nh * c.HD))

    wq_p = permute_heads(wq).astype(BF16)
    wk_p = permute_heads(wk).astype(BF16)
    wv_b = wv.astype(BF16)
    wo_b = wo.astype(BF16)

    # rope tables, even/odd-split feature-major layout: [128, T]
    hh = c.HD // 2
    inv = 1.0 / (c.THETA ** (np.arange(0, c.HD, 2, dtype=np.float64) / c.HD))
    pos = (np.arange(c.T) % c.S).astype(np.float64)
    ang = inv[:, None] * pos[None, :]              # [64, T]
    cosv = np.cos(ang).astype(np.float32)
    sinv = np.sin(ang).astype(np.float32)
    cosi = np.concatenate([cosv, cosv], 0)
    sini = np.concatenate([-sinv, sinv], 0)
    assert hh * 2 == P

    trilm = np.ascontiguousarray(
        np.tril(np.ones((P, P), np.float32)).T
    ).astype(BF16)                                  # [k, q]: 1 iff k<=q
    identm = np.eye(P, dtype=np.float32)

    KHC = c.KVH // c.NCORES  # kv heads per core (=1)
    in_maps = []
    for cc in range(c.NCORES):
        in_maps.append({
            "xT": xT,
            "wq": np.ascontiguousarray(wq_p[:, cc * c.QF:(cc + 1) * c.QF]),
            "wk": np.ascontiguousarray(
                wk_p[:, cc * KHC * c.HD:(cc * KHC + 1) * c.HD]),
            "wv": np.ascontiguousarray(
                wv_b[:, cc * KHC * c.HD:(cc * KHC + 1) * c.HD]),
            "wo": np.ascontiguousarray(wo_b[cc * c.QF:(cc + 1) * c.QF, :]),
            "cosi": cosi,
            "sini": sini,
            "tril": trilm,
            "ident": identm,
        })
    return in_maps


def assemble(c, outs):
    """outs[c]: [NCH*OSH, DIM] token shards; group g rows [g*OSH, (g+1)*OSH)
    of core r are global tokens [g*TCH + r*OSH, +OSH)."""
    full = np.empty((c.T, c.DIM), np.float32)
    for g in range(c.NCH):
        for r in range(c.NCORES):
            t0 = g * c.TCH + r * c.OSH
            full[t0:t0 + c.OSH] = np.asarray(
                outs[r][g * c.OSH:(g + 1) * c.OSH]).astype(np.float32)
    return full.reshape(c.B, c.S, c.DIM)


def kernel(x, wq, wk, wv, wo):
    from concourse import bass_utils

    if "nc" not in _CACHE:
        _CACHE["cfg"] = make_cfg()
        _CACHE["nc"] = _build_graph(_CACHE["cfg"])
    nc = _CACHE["nc"]
    c = _CACHE["cfg"]

    in_maps = _host_inputs(
        c, np.asarray(x), np.asarray(wq), np.asarray(wk),
        np.asarray(wv), np.asarray(wo),
    )
    res = bass_utils.run_bass_kernel_spmd(
        nc, in_maps, core_ids=list(range(c.NCORES)), trace=_TRACE
    )
    _CACHE["last_results"] = res
    outs = [res.results[i]["out"] for i in range(c.NCORES)]
    return assemble(c, outs)


# revision 85
# speedup vs baseline: 1.0137x; 1.0137x over previous
"""
Distributed GQA attention block for Trainium2 (8 NeuronCores).

Problem: out = AttentionBlock(x; wq, wk, wv, wo)
  B=2, S=2048, DIM=4096, n_heads=32, n_kv_heads=8, head_dim=128,
  rope theta=5e5, causal, softmax, f32 I/O.

Sharding strategy (tensor-parallel over heads, pipelined ReduceScatter
after a ROW-parallel output projection):
  - Each core c owns 4 query heads (4c..4c+3) and 1 kv head (c).
  - Per core: q/k/v projections for its heads (column shards of wq/wk/wv),
    RoPE, causal attention for its 4 heads over the full sequence. The
    per-core attention output stays in SBUF, stored transposed
    [512 feat, tokens] in bf16.
  - The output projection is ROW-sharded: core c computes
    partial = attnT_c^T @ wo[512c:512c+512, :]  -> [tokens, 4096] bf16,
    streamed to DRAM in 512-token groups. Each group is ReduceScattered
    (sum over the 8 cores, scatter over tokens) into the kernel output:
    core c receives tokens [g*512 + 64c : g*512 + 64c + 64) of group g.
    Host-side unshard is pure index assembly along tokens (no compute).
  - The 8 small ReduceScatters pipeline behind the projection compute, so
    only the last (~28us) is exposed, vs ~330us of exposed AllGather time
    in the gather + column-parallel-wo formulation.

Compute dtype: bf16 operands with f32 PSUM accumulation. Softmax skips
the max-subtraction (scores are < ~15 at this problem's scale), the
denominator comes free from an appended ones-column in the PV matmul,
and normalization is applied to the [tok, 128] attention output instead
of the [tok, 2048] probabilities.

RoPE layout trick: wq/wk columns are host-permuted so each head's even
dims come first and odd dims second. The rotation's pair swap then
becomes two 64-partition block copies (SBUF->SBUF DMA) instead of a
cross-partition interleave.
"""

import math
from types import SimpleNamespace

import numpy as np
import ml_dtypes

P = 128
BF16 = ml_dtypes.bfloat16


_CACHE = {}
_TRACE = False


def make_cfg(B=2, S=2048, DIM=4096, H=32, KVH=8, HD=128, THETA=500000.0,
             NCORES=8):
    c = SimpleNamespace(B=B, S=S, DIM=DIM, H=H, KVH=KVH, HD=HD, THETA=THETA,
                        NCORES=NCORES)
    c.T = B * S
    c.HPC = H // NCORES          # query heads per core
    c.QF = c.HPC * HD            # query features per core
    c.SCALE = 1.0 / math.sqrt(HD)
    c.TCH = 512                  # token chunk
    c.NKT = DIM // P             # contraction tiles
    c.NTT = c.T // P             # token tiles
    c.NCH = c.T // c.TCH         # token chunks (= ReduceScatter groups)
    c.SQT = S // P               # q/k tiles per sequence
    c.VW = HD + 1                # v + ones column
    c.OSH = c.TCH // NCORES      # output token rows per core per group
    assert S % c.TCH == 0 and c.T % c.TCH == 0 and DIM % P == 0
    # each core's HPC query heads share the core's single kv head
    assert KVH == NCORES and c.HPC == H // KVH
    return c


def _build_graph(c, phases=4):
    """Build + compile the SPMD Bass graph (same program on every core)."""
    import concourse.mybir as mybir
    import concourse.tile as tile
    from concourse import bacc

    fp32 = mybir.dt.float32
    bf16 = mybir.dt.bfloat16

    nc = bacc.Bacc(
        "TRN2",
        target_bir_lowering=False,
        debug=False,
        enable_asserts=True,
        num_devices=c.NCORES,
    )

    # ---- kernel I/O ----
    xT = nc.dram_tensor("xT", [c.DIM, c.T], bf16, kind="ExternalInput").ap()
    wq = nc.dram_tensor("wq", [c.DIM, c.QF], bf16, kind="ExternalInput").ap()
    wk = nc.dram_tensor("wk", [c.DIM, c.HD], bf16, kind="ExternalInput").ap()
    wv = nc.dram_tensor("wv", [c.DIM, c.HD], bf16, kind="ExternalInput").ap()
    wo = nc.dram_tensor("wo", [c.QF, c.DIM], bf16, kind="ExternalInput").ap()
    cosi = nc.dram_tensor("cosi", [P, c.T], fp32, kind="ExternalInput").ap()
    sini = nc.dram_tensor("sini", [P, c.T], fp32, kind="ExternalInput").ap()
    tril = nc.dram_tensor("tril", [P, P], bf16, kind="ExternalInput").ap()
    ident = nc.dram_tensor("ident", [P, P], fp32, kind="ExternalInput").ap()
    # token-sharded output: group g of 512 tokens contributes rows
    # [g*64, g*64+64) = global tokens [g*512 + 64*rank, +64)
    out = nc.dram_tensor("out", [c.NCH * c.OSH, c.DIM], bf16,
                         kind="ExternalOutput").ap()

    Exp = mybir.ActivationFunctionType.Exp
    Copy = mybir.ActivationFunctionType.Copy
    TPP = c.TCH // P          # token sub-tiles per chunk
    NQT = c.HPC + 1           # rope targets per chunk: HPC q tiles + 1 k
    SPB = c.S // P            # 128-token tiles per batch
    CPB = c.NCH // c.B        # token chunks per batch
    KG = 4                    # contraction tiles fetched per DMA
    NOC = c.DIM // c.TCH      # output column chunks (phase 4)

    with tile.TileContext(nc) as tc:
        # ------- static SBUF tensors (split per batch so attention of
        # batch 0 can start while batch 1 is still projecting) -------
        qT_b, kT_b, v_b, free_stat = [], [], [], []
        for b in range(c.B):
            t_, f_ = tc.tile([P, c.HPC, c.S], bf16, name=f"qT_sb{b}")
            qT_b.append(t_); free_stat.append(f_)
            t_, f_ = tc.tile([P, c.S], bf16, name=f"kT_sb{b}")
            kT_b.append(t_); free_stat.append(f_)
            t_, f_ = tc.tile([P, SPB, c.VW], bf16, name=f"v_sb{b}")
            v_b.append(t_); free_stat.append(f_)
        tril_sb, free_tril = tc.tile([P, P], bf16, name="tril_sb")
        id_sb, free_id = tc.tile([P, P], fp32, name="id_sb")
        idb_sb, free_idb = tc.tile([P, P], bf16, name="idb_sb")
        free_stat += [free_tril, free_id, free_idb]

        for b in range(c.B):
            nc.vector.memset(v_b[b][:, :, c.HD:c.VW], 1.0)  # denominator ones

        # dummy exp at t=0: pulls the ~2.7us exp_and_others ACT-table load
        # off the attention critical path (Copy is filler in every set, so
        # no switch-back occurs later). Sourced from the memset region so it
        # does not wait on any DMA.
        warm_sb, free_warm = tc.tile([1, 1], fp32, name="warm_sb")
        nc.scalar.activation(warm_sb[:], v_b[0][0:1, 0, c.HD:c.HD + 1], Exp)
        free_stat.append(free_warm)

        with tc.tile_pool(name="dram", bufs=1, space="DRAM") as dramp:
            # per-group partial output-projection products (RS inputs)
            part_g = [
                dramp.tile([c.TCH, c.DIM], bf16, name=f"part{g}")
                for g in range(c.NCH)
            ]
            # RS outputs (collectives may not write IO tensors directly)
            rs_g = [
                dramp.tile([c.OSH, c.DIM], bf16, name=f"rs{g}")
                for g in range(c.NCH)
            ]

            # ============ Phase 1: projections + RoPE ============
            WG = 8                    # wk/wv contraction tiles per DMA
            with tc.tile_pool(name="wpool", bufs=1) as wpool, \
                 tc.tile_pool(name="xpool", bufs=3) as xpool, \
                 tc.tile_pool(name="tabs", bufs=2) as tabs, \
                 tc.tile_pool(name="rope", bufs=2) as ropep, \
                 tc.tile_pool(name="pj_ps", bufs=1, space="PSUM") as pjps:

                # per-kt weight tiles: wq rides the fast sync/HWDGE queue
                # interleaved just-in-time with the x stream (4-tile
                # mega-DMAs); wk/wv go on the gpsimd queue in 8-tile
                # mega-DMAs (64 separate small DMAs can't keep up with the
                # matmul stream's 1.28us/kt)
                wqb = [None] * (c.NKT // KG)
                wkb, wvb = [], []
                for gi in range(c.NKT // WG):
                    r0, r1 = gi * WG * P, (gi + 1) * WG * P
                    wkt = wpool.tile([P, WG, c.HD], bf16, tag="wk",
                                     bufs=c.NKT // WG, name=f"wk_g{gi}")
                    wvt = wpool.tile([P, WG, c.HD], bf16, tag="wv",
                                     bufs=c.NKT // WG, name=f"wv_g{gi}")
                    if gi == 0:
                        # split head: the first 2 k/v tiles land fast so
                        # the t=0 matmuls are not gated on an 8-tile DMA
                        rm = r0 + 2 * P
                        nc.gpsimd.dma_start(
                            wkt[:, 0:2, :],
                            wk[r0:rm, :].rearrange("(o p) h -> p o h", p=P))
                        nc.gpsimd.dma_start(
                            wvt[:, 0:2, :],
                            wv[r0:rm, :].rearrange("(o p) h -> p o h", p=P))
                        nc.gpsimd.dma_start(
                            wkt[:, 2:WG, :],
                            wk[rm:r1, :].rearrange("(o p) h -> p o h", p=P))
                        nc.gpsimd.dma_start(
                            wvt[:, 2:WG, :],
                            wv[rm:r1, :].rearrange("(o p) h -> p o h", p=P))
                    else:
                        nc.gpsimd.dma_start(
                            wkt[:],
                            wk[r0:r1, :].rearrange("(o p) h -> p o h", p=P))
                        nc.gpsimd.dma_start(
                            wvt[:],
                            wv[r0:r1, :].rearrange("(o p) h -> p o h", p=P))
                    wkb.append(wkt)
                    wvb.append(wvt)

                def wk_at(kt):
                    return wkb[kt // WG][:, kt % WG, :]

                def wv_at(kt):
                    return wvb[kt // WG][:, kt % WG, :]

                def load_wq(kg):
                    wqt = wpool.tile([P, KG, c.QF], bf16, tag="wq",
                                     bufs=c.NKT // KG, name=f"wq_g{kg}")
                    r0 = kg * KG * P
                    if kg == 0:
                        nc.sync.dma_start(
                            wqt[:, 0:1, :],
                            wq[r0:r0 + P, :].rearrange(
                                "(o p) f -> p o f", p=P))
                        nc.sync.dma_start(
                            wqt[:, 1:KG, :],
                            wq[r0 + P:r0 + KG * P, :].rearrange(
                                "(o p) f -> p o f", p=P))
                    else:
                        nc.sync.dma_start(
                            wqt[:],
                            wq[r0:r0 + KG * P, :].rearrange(
                                "(o p) f -> p o f", p=P))
                    wqb[kg] = wqt

                def wq_at(kt, ft):
                    return wqb[kt // KG][:, kt % KG, ft * P:(ft + 1) * P]

                for ch in range(c.NCH):
                    t0 = ch * c.TCH
                    bch = ch // CPB           # batch of this chunk
                    lt0 = t0 - bch * c.S      # batch-local token offset
                    q_ps = [
                        pjps.tile([P, c.TCH], fp32, tag=f"q{ft}", bufs=1,
                                  name=f"q_ps{ft}")
                        for ft in range(c.HPC)
                    ]
                    k_ps = pjps.tile([P, c.TCH], fp32, tag="k", bufs=1)
                    v_ps = pjps.tile([P, TPP, P], fp32, tag="v", bufs=1)

                    for kg in range(c.NKT // KG):
                        # one DMA brings KG=4 contraction tiles (512 KB)
                        xt4 = xpool.tile([P, KG, c.TCH], bf16, tag="xt")
                        r0 = kg * KG * P
                        if ch == 0 and kg == 0:
                            # split head for a fast t=0 start
                            nc.sync.dma_start(
                                xt4[:, 0:1, :],
                                xT[r0:r0 + P, t0:t0 + c.TCH].rearrange(
                                    "(o p) t -> p o t", p=P))
                            nc.sync.dma_start(
                                xt4[:, 1:KG, :],
                                xT[r0 + P:r0 + KG * P,
                                   t0:t0 + c.TCH].rearrange(
                                    "(o p) t -> p o t", p=P))
                        else:
                            nc.sync.dma_start(
                                xt4[:],
                                xT[r0:r0 + KG * P, t0:t0 + c.TCH].rearrange(
                                    "(o p) t -> p o t", p=P))
                        if ch == 0:
                            # wq arrives just behind the x tile it multiplies
                            load_wq(kg)
                        if ch == 0 and kg == 0:
                            # small static loads ride behind the first
                            # wq/x tiles so the first matmul starts sooner
                            nc.sync.dma_start(tril_sb[:], tril[:])
                            nc.sync.dma_start(id_sb[:], ident[:])
                            nc.vector.tensor_copy(idb_sb[:], id_sb[:])
                        for ki in range(KG):
                            kt = kg * KG + ki
                            xt = xt4[:, ki, :]
                            st = kt == 0
                            sp = kt == c.NKT - 1

                            def mm_kv():
                                nc.tensor.matmul(
                                    k_ps[:], lhsT=wk_at(kt), rhs=xt,
                                    start=st, stop=sp,
                                )
                                # v TOKEN-major: the x tile is lhsT, so no
                                # PE transposes are needed afterwards (same
                                # row count: 4x128 free vs 1x512)
                                for sub in range(TPP):
                                    # start zeroes the whole PSUM bank, so
                                    # only the first of the four region
                                    # chains may issue it
                                    nc.tensor.matmul(
                                        v_ps[:, sub, :],
                                        lhsT=xt4[:, ki,
                                                 sub * P:(sub + 1) * P],
                                        rhs=wv_at(kt),
                                        start=(st and sub == 0), stop=sp,
                                    )

                            if ch == 0:
                                # k/v weights land first (gpsimd queue), so
                                # at t=0 their matmuls lead the q ones
                                mm_kv()
                            for ft in range(c.HPC):
                                nc.tensor.matmul(
                                    q_ps[ft][:],
                                    lhsT=wq_at(kt, ft),
                                    rhs=xt,
                                    start=st, stop=sp,
                                )
                            if ch != 0:
                                mm_kv()

                    # ---- RoPE on all q tiles + k at once (mega-tile) ----
                    ct = tabs.tile([P, c.TCH], fp32, tag="cos")
                    st_t = tabs.tile([P, c.TCH], fp32, tag="sin")
                    nc.sync.dma_start(ct[:], cosi[:, t0:t0 + c.TCH])
                    nc.sync.dma_start(st_t[:], sini[:, t0:t0 + c.TCH])

                    qbig = ropep.tile([P, NQT, c.TCH], fp32, tag="qbig",
                                      name="qbig")
                    # psum -> sbuf copies split across ACT and DVE, ordered
                    # by the next chunk's accumulator-reuse order (q0 first,
                    # v last) so the next chunk's matmuls never wait:
                    #   ACT: q0, q2, k, v0, v2     DVE: q1, q3, v1, v3
                    gt0 = lt0 // P
                    nc.scalar.activation(qbig[:, 0, :], q_ps[0][:], Copy)
                    nc.vector.tensor_copy(qbig[:, 1, :], q_ps[1][:])
                    nc.scalar.activation(qbig[:, 2, :], q_ps[2][:], Copy)
                    nc.vector.tensor_copy(qbig[:, 3, :], q_ps[3][:])
                    nc.scalar.activation(qbig[:, c.HPC, :], k_ps[:], Copy)
                    nc.scalar.activation(v_b[bch][:, gt0, 0:c.HD],
                                         v_ps[:, 0, :], Copy)
                    nc.vector.tensor_copy(v_b[bch][:, gt0 + 1, 0:c.HD],
                                          v_ps[:, 1, :])
                    nc.scalar.activation(v_b[bch][:, gt0 + 2, 0:c.HD],
                                         v_ps[:, 2, :], Copy)
                    nc.vector.tensor_copy(v_b[bch][:, gt0 + 3, 0:c.HD],
                                          v_ps[:, 3, :])

                    qsw = ropep.tile([P, NQT, c.TCH], fp32, tag="qsw",
                                     name="qsw")
                    # pair swap == half-partition block swap (even|odd split)
                    nc.sync.dma_start(qsw[0:64, :, :], qbig[64:128, :, :])
                    nc.sync.dma_start(qsw[64:128, :, :], qbig[0:64, :, :])

                    ctb = ct[:, None, :].to_broadcast((P, NQT, c.TCH))
                    stb = st_t[:, None, :].to_broadcast((P, NQT, c.TCH))
                    eng = nc.vector
                    eng.tensor_mul(qbig[:], qbig[:], ctb)
                    eng.tensor_mul(qsw[:], qsw[:], stb)
                    rr = ropep.tile([P, NQT, c.TCH], bf16, tag="rr", name="rr")
                    eng.tensor_add(rr[:], qbig[:], qsw[:])
                    for ft in range(c.HPC):
                        eng.tensor_copy(
                            qT_b[bch][:, ft, lt0:lt0 + c.TCH], rr[:, ft, :])
                    eng.tensor_copy(
                        kT_b[bch][:, lt0:lt0 + c.TCH], rr[:, c.HPC, :])


            # ====== Phases 2+4 per batch: attention -> wo -> RS ======
            if phases >= 2:
                with tc.tile_pool(name="spool", bufs=3, space="PSUM") as spool, \
                     tc.tile_pool(name="opool", bufs=4, space="PSUM") as opool, \
                     tc.tile_pool(name="tpool", bufs=1, space="PSUM") as tpool, \
                     tc.tile_pool(name="ppool", bufs=32) as ppool, \
                     tc.tile_pool(name="apool", bufs=4) as apool, \
                     tc.tile_pool(name="wop", bufs=1) as wop, \
                     tc.tile_pool(name="atp", bufs=2) as atp, \
                     tc.tile_pool(name="obp", bufs=2) as obp:

                    # wo row-shard [QF, DIM] as 4 feature-partition tiles;
                    # loaded on the gpsimd queue during batch-0 attention
                    wo_t = []
                    for f in range(c.HPC):
                        wot = wop.tile([P, c.DIM], bf16, tag="wo",
                                       bufs=c.HPC, name=f"wo_t{f}")
                        nc.gpsimd.dma_start(wot[:], wo[f * P:(f + 1) * P, :])
                        wo_t.append(wot)

                    def finalize(pend):
                        # PE transpose of the normalized attention tile +
                        # copy into the SBUF attnT store. Deferred one step
                        # behind the main loop so the transpose never waits
                        # on the DVE normalize chain. Copies go to DVE: the
                        # ACT engine is saturated by the exp stream.
                        p_attnT, p_h, p_qi, p_ao = pend
                        tp = tpool.tile([P, P], bf16, tag="tp", name="tp")
                        nc.tensor.transpose(tp[:], p_ao[:], idb_sb[:])
                        nc.vector.tensor_copy(
                            p_attnT[:, p_h, p_qi * P:(p_qi + 1) * P], tp[:])

                    def emit_last_pv(pend):
                        # deferred final PV accumulation steps (the ones
                        # consuming the freshest exp outputs) + normalize.
                        # Emitted after the NEXT unit's scores so PE work
                        # covers the ACT exp latency.
                        p_ops, p_steps, p_start, p_qi, p_attnT, p_h = pend
                        for i, (p_pt, p_off, p_vt) in enumerate(p_steps):
                            nc.tensor.matmul(
                                p_ops, lhsT=p_pt[:, p_off:p_off + P],
                                rhs=p_vt,
                                start=(p_start and i == 0),
                                stop=(i == len(p_steps) - 1),
                            )
                        rec = apool.tile([P, 1], fp32, tag="rec", name="rec")
                        nc.vector.reciprocal(rec[:], p_ops[:, c.HD:c.VW])
                        # normalize + cast bf16 in one DVE op; the bf16
                        # operand makes the PE transpose 1 cyc/row
                        ao = apool.tile([P, P], bf16, tag="ao", name="ao")
                        nc.vector.tensor_scalar_mul(
                            ao[:], p_ops[:, 0:c.HD], rec[:]
                        )
                        return (p_attnT, p_h, p_qi, ao)

                    pending_o = None
                    pend_pv = None
                    # single-bank ping-pong PV accumulator (region-tracked)
                    o_pp = opool.tile([P, 2, c.VW], fp32, tag="o", bufs=1,
                                      name="o_pp")
                    n_ki = 0
                    SC = 1024             # score tile width (one exp per tile)
                    def score_tile(b, qh, kj, c0, w, tril):
                        s_ps = spool.tile([P, SC], fp32, tag="s",
                                          name="s_ps")
                        for sub in range(0, w, c.TCH):
                            ws = min(c.TCH, w - sub)
                            nc.tensor.matmul(
                                s_ps[:, sub:sub + ws],
                                lhsT=kT_b[b][:, kj * P:(kj + 1) * P],
                                rhs=qh[:, c0 + sub:c0 + sub + ws],
                                start=True, stop=True,
                            )
                        pt = ppool.tile([P, SC], bf16, tag="pt", bufs=28,
                                        name="pt")
                        nc.scalar.activation(pt[:, :w], s_ps[:, :w], Exp,
                                             scale=c.SCALE)
                        if tril:
                            nc.vector.tensor_mul(pt[:, 0:P], pt[:, 0:P],
                                                 tril_sb[:])
                        return pt

                    def emit_unit(b, h, ki, attnT, pts):
                        # one (head, k-tile) attention step: fresh scores,
                        # previous deferred work, this qi's PV chain
                        nonlocal pending_o, pend_pv, n_ki
                        qh = qT_b[b][:, h, :]
                        q0 = ki * P
                        nmc = (c.S - q0 + SC - 1) // SC
                        for mc in range(nmc):
                            c0 = q0 + mc * SC
                            pts[(ki, mc)] = score_tile(
                                b, qh, ki, c0, min(SC, c.S - c0), mc == 0)
                        if pend_pv is not None:
                            old_fin = pending_o
                            pending_o = emit_last_pv(pend_pv)
                            pend_pv = None
                            if old_fin is not None:
                                finalize(old_fin)
                        # ---- PV for qi == ki (all but the last step;
                        # that one is deferred past the next scores) ----
                        qi = ki
                        o_ps = o_pp[:, n_ki % 2, :]
                        n_ki += 1
                        for kj in range(qi):
                            qoff = (qi - kj) * P
                            mc = qoff // SC
                            off = qoff % SC
                            nc.tensor.matmul(
                                o_ps,
                                lhsT=pts[(kj, mc)][:, off:off + P],
                                rhs=v_b[b][:, kj, :],
                                start=(kj == 0), stop=False,
                            )
                        pend_pv = (o_ps,
                                   [(pts[(qi, 0)], 0, v_b[b][:, qi, :])],
                                   qi == 0, qi, attnT, h)

                    attnTs = [
                        atp.tile([P, c.HPC, c.S], bf16, tag="attnT",
                                 name=f"attnT{bb}")
                        for bb in range(c.B)
                    ]
                    pts_next = {}  # batch-1 head-0 tiles (built inside P4 b0)
                    pts_next2 = {}  # batch-1 head-1 tiles (ditto)
                    pts_next3 = {}  # batch-1 head-2 tiles (ditto)

                    def p4_batch(b, attnT, extra):
                        # row-parallel wo + pipelined RS; `extra` holds one
                        # attention-unit thunk per token tile (the next
                        # batch's first head rides the idle ACT engine here)
                        nonlocal pending_o, pend_pv
                        ei = 0
                        for gl in range(CPB):         # 512-token groups
                            g = b * CPB + gl
                            for tt in range(TPP):     # 128-token tiles
                                lt = gl * c.TCH + tt * P
                                ob = obp.tile([P, c.DIM], bf16, tag="ob",
                                              bufs=3, name="ob")
                                for cc in range(c.DIM // SC):
                                    o4 = spool.tile([P, SC], fp32,
                                                    tag="s", name="o4_ps")
                                    for half in range(SC // c.TCH):
                                        hb = half * c.TCH
                                        for f in range(c.HPC):
                                            nc.tensor.matmul(
                                                o4[:, hb:hb + c.TCH],
                                                lhsT=attnT[:, f, lt:lt + P],
                                                rhs=wo_t[f][:,
                                                            cc * SC + hb:
                                                            cc * SC + hb
                                                            + c.TCH],
                                                start=(f == 0),
                                                stop=(f == c.HPC - 1),
                                            )
                                    if pend_pv is not None:
                                        old_fin = pending_o
                                        pending_o = emit_last_pv(pend_pv)
                                        pend_pv = None
                                        if old_fin is not None:
                                            finalize(old_fin)
                                    elif pending_o is not None:
                                        finalize(pending_o)
                                        pending_o = None
                                    dst = ob[:, cc * SC:(cc + 1) * SC]
                                    if cc % 2 == 0:
                                        nc.scalar.activation(dst, o4[:], Copy)
                                    else:
                                        nc.vector.tensor_copy(dst, o4[:])
                                # on the gpsimd dma queue: the sync queue is
                                # reserved for rs->out copies (which wait on
                                # a RS) and the ACT/DVE queues must keep
                                # draining PSUM copies, so partial writes
                                # never block anything
                                nc.gpsimd.dma_start(
                                    part_g[g][tt * P:(tt + 1) * P, :], ob[:]
                                )
                                for _ in range(3):
                                    if ei < len(extra):
                                        extra[ei]()
                                        ei += 1
                            if phases >= 3:
                                # sum over cores, scatter over this group's
                                # tokens: rank r receives rows [64r, 64r+64)
                                nc.gpsimd.collective_compute(
                                    "ReduceScatter",
                                    mybir.AluOpType.add,
                                    replica_groups=[list(range(c.NCORES))],
                                    ins=[part_g[g][:].opt()],
                                    outs=[rs_g[g][:].opt()],
                                )
                                nc.sync.dma_start(
                                    out[g * c.OSH:(g + 1) * c.OSH, :],
                                    rs_g[g][:],
                                )
                        assert ei == len(extra)

                    for b in range(c.B):
                        # -------- attention for this batch's heads -------
                        # (batch 1's head 0 already ran inside P4 of b0)
                        attnT = attnTs[b]
                        for h in range(3 if b > 0 else 0, c.HPC):
                            pts = {}
                            for ki in range(c.SQT):
                                emit_unit(b, h, ki, attnT, pts)
                        if b == 0:
                            extra = [
                                (lambda kk=kk, hh=hh, pp=pp: emit_unit(
                                    1, hh, kk, attnTs[1], pp))
                                for hh, pp in ((0, pts_next), (1, pts_next2),
                                               (2, pts_next3))
                                for kk in range(c.SQT)
                            ]
                        else:
                            extra = []
                        p4_batch(b, attnT, extra)

        # release static single-tile pools in LIFO order
        for f_ in reversed(free_stat):
            f_()

    nc.compile()
    return nc


def _host_inputs(c, x, wq, wk, wv, wo):
    """Shard + lay out the inputs for the cores."""
    xT = np.ascontiguousarray(x.reshape(c.T, c.DIM).T).astype(BF16)

    # even/odd split permutation within each head (q and k only)
    perm_head = np.concatenate([np.arange(0, c.HD, 2), np.arange(1, c.HD, 2)])

    def permute_heads(w):  # w: [DIM, n*HD]
        nh = w.shape[1] // c.HD
        w = w.reshape(c.DIM, nh, c.HD)[:, :, perm_head]
        return np.ascontiguousarray(w.reshape(c.DIM, nh * c.HD))

    wq_p = permute_heads(wq).astype(BF16)
    wk_p = permute_heads(wk).astype(BF16)
    wv_b = wv.astype(BF16)
    wo_b = wo.astype(BF16)

    # rope tables, even/odd-split feature-major layout: [128, T]
    hh = c.HD // 2
    inv = 1.0 / (c.THETA ** (np.arange(0, c.HD, 2, dtype=np.float64) / c.HD))
    pos = (np.arange(c.T) % c.S).astype(np.float64)
    ang = inv[:, None] * pos[None, :]              # [64, T]
    cosv = np.cos(ang).astype(np.float32)
    sinv = np.sin(ang).astype(np.float32)
    cosi = np.concatenate([cosv, cosv], 0)
    sini = np.concatenate([-sinv, sinv], 0)
    assert hh * 2 == P

    trilm = np.ascontiguousarray(
        np.tril(np.ones((P, P), np.float32)).T
    ).astype(BF16)                                  # [k, q]: 1 iff k<=q
    identm = np.eye(P, dtype=np.float32)

    KHC = c.KVH // c.NCORES  # kv heads per core (=1)
    in_maps = []
    for cc in range(c.NCORES):
        in_maps.append({
            "xT": xT,
            "wq": np.ascontiguousarray(wq_p[:, cc * c.QF:(cc + 1) * c.QF]),
            "wk": np.ascontiguousarray(
                wk_p[:, cc * KHC * c.HD:(cc * KHC + 1) * c.HD]),
            "wv": np.ascontiguousarray(
                wv_b[:, cc * KHC * c.HD:(cc * KHC + 1) * c.HD]),
            "wo": np.ascontiguousarray(wo_b[cc * c.QF:(cc + 1) * c.QF, :]),
            "cosi": cosi,
            "sini": sini,
            "tril": trilm,
            "ident": identm,
        })
    return in_maps


def assemble(c, outs):
    """outs[c]: [NCH*OSH, DIM] token shards; group g rows [g*OSH, (g+1)*OSH)
    of core r are global tokens [g*TCH + r*OSH, +OSH)."""
    full = np.empty((c.T, c.DIM), np.float32)
    for g in range(c.NCH):
        for r in range(c.NCORES):
            t0 = g * c.TCH + r * c.OSH
            full[t0:t0 + c.OSH] = np.asarray(
                outs[r][g * c.OSH:(g + 1) * c.OSH]).astype(np.float32)
    return full.reshape(c.B, c.S, c.DIM)


def kernel(x, wq, wk, wv, wo):
    from concourse import bass_utils

    if "nc" not in _CACHE:
        _CACHE["cfg"] = make_cfg()
        _CACHE["nc"] = _build_graph(_CACHE["cfg"])
    nc = _CACHE["nc"]
    c = _CACHE["cfg"]

    in_maps = _host_inputs(
        c, np.asarray(x), np.asarray(wq), np.asarray(wk),
        np.asarray(wv), np.asarray(wo),
    )
    res = bass_utils.run_bass_kernel_spmd(
        nc, in_maps, core_ids=list(range(c.NCORES)), trace=_TRACE
    )
    _CACHE["last_results"] = res
    outs = [res.results[i]["out"] for i in range(c.NCORES)]
    return assemble(c, outs)
